# revision 28
# baseline (speedup 1.0000x reference)
"""CenterLoss kernel for Trainium2 (8 NeuronCores, SPMD data-parallel).

Math: for pixel p with feature x_p (256-ch), label l_p, centers C[19,256]:
    dist_p = ||xn_p||^2 + ||cn_{l_p}||^2 - 2 * xn_p . cn_{l_p}
with xn = x/||x||, cn = C/||C|| (row-wise).  ||xn||^2 == ||cn||^2 == 1 up to
f32 rounding, so   mean(dist) = 2 - (2/B) * S,  S = sum_p (x_p.cn_{l_p})/||x_p||.

Device pipeline (per 2048-px tile, 4 col-tiled 512-px groups g in parallel
32-column strips of the PE array -- every output here is <=32 partitions
wide, so 4 matmuls with different moving operands run concurrently):
  - dots4[32g+k, p] = ct.T @ x_g            (PE, strip g; chunk0 fp8 +
                     chunk1 bf16 accumulated in PSUM f32)
  - prodsel4       = onehot4 * dots4        (DVE, one op per tile; onehot4
                     is staged host-side in the same 32g+k partition layout)
  - sel[32g+u, p] += rc-col.T @ prodsel4_g  (PE, strip (g,g); u = tile parity;
                     stationary [19,2] col u = rc = 1/||C_k||)
  - ss[32g+u, p]  += ones-col.T @ xsq_g     (PE, strip g)
  - xsq: three engines in parallel per tile: ACT Square on fp8 chunk0 halves
    0:1024, GPSIMD mul on chunk0 1024:2048, DVE mul on bf16 chunk1 (2x mode)
  - finish per bank (tiles 01 -> bank A, 23 -> B, A overlapped with compute):
    out = sum_p sel * rsqrt(ss)   (ACT Abs_reciprocal_sqrt; same act table
    set as Square so only one ACT_TABLE_LOAD)

PSUM has_written discipline: start=True clears the WHOLE bank's bits, so
each bank gets exactly one start=True on its first matmul; all later
matmuls use flags=0 (overwrite-if-clear / accumulate-if-set per element).

DMA: x chunk0 (fp8, 1MiB) tiles on the sync HWDGE ring; onehot4 + x chunk1
(bf16, 2MiB) tiles on the scalar HWDGE ring (parallel issue).  Everything
is staged partition-major contiguous (>=2KB per-partition runs).

Sharding: 65536 pixels -> 8 cores x 8192 (core c: image c//2, half c%2).
Host sums the live partitions (32g+u) of the [2,128,1] output.
"""

import sys

import numpy as np

if "/opt/trn_rl_repo" not in sys.path:
    sys.path.insert(0, "/opt/trn_rl_repo")

import concourse.bacc as bacc
import concourse.tile as tile
from concourse import mybir
from concourse.bass_utils import run_bass_kernel_spmd

N_CORES = 8
C = 256
NCLS = 19
N_IMG, H, W = 4, 128, 128
PIX_TOTAL = N_IMG * H * W          # 65536
PIX_PER_CORE = PIX_TOTAL // N_CORES  # 8192
TILE_F = 2048                      # pixels per DMA tile
N_TILES = PIX_PER_CORE // TILE_F   # 4
GRP = 512                          # pixels per col-strip group
F32 = mybir.dt.float32
BF16 = mybir.dt.bfloat16
FP8 = mybir.dt.float8e4


def build_nc():
    """Build the per-core Bass program (same program on all 8 cores)."""
    AF = mybir.ActivationFunctionType

    import ml_dtypes

    nc = bacc.Bacc(None, target_bir_lowering=False, debug=False)
    x_d = nc.dram_tensor("x", [N_TILES, 128, 2, TILE_F], BF16, kind="ExternalInput")
    oh_d = nc.dram_tensor("onehot4", [128, N_TILES, GRP], FP8, kind="ExternalInput")
    ct_d = nc.dram_tensor("centersT", [128, 2, NCLS], BF16, kind="ExternalInput")
    out_d = nc.dram_tensor("out", [2, 128, 1], F32, kind="ExternalOutput")

    with tile.TileContext(nc) as tc:
        with (
            tc.tile_pool(name="consts", bufs=1) as consts,
            tc.tile_pool(name="x0in", bufs=4) as x0in,
            tc.tile_pool(name="x1in", bufs=4) as x1in,
            tc.tile_pool(name="xsq", bufs=2) as xsqp,
            tc.tile_pool(name="small", bufs=2) as small,
            tc.tile_pool(name="accum", bufs=1) as accp,
            tc.tile_pool(name="dots", bufs=3, space="PSUM") as dotsp,
            tc.tile_pool(name="acc_ps", bufs=1, space="PSUM") as accps,
        ):
            # ---- DMAs: byte-balanced across the two HWDGE rings in tile
            # consumption order (each ring is one FIFO queue; the SDMA
            # engines round-robin between the two queues) ----
            ct_in = consts.tile([128, 2, NCLS], BF16, tag="ct_in")
            nc.sync.dma_start(out=ct_in[:], in_=ct_d[:])
            oh4 = consts.tile([128, N_TILES, GRP], FP8, tag="oh4")
            nc.gpsimd.dma_start(out=oh4[:], in_=oh_d[:])
            xts = []
            for t in range(N_TILES):
                xt = x0in.tile([128, 2, TILE_F], BF16, tag="xt", name=f"xt{t}")
                ring = nc.sync if t % 2 == 0 else nc.gpsimd
                ring.dma_start(out=xt[:], in_=x_d[t])
                xts.append(xt)

            # ---- constants ----
            # wmov first: the PE warmup matmuls depend only on it.
            wmov = consts.tile([128, GRP], BF16, tag="wmov")
            nc.vector.memset(wmov[:], 0.5)
            ones_b = consts.tile([128, 1], BF16, tag="ones_b")
            nc.vector.memset(ones_b[:], 1.0)
            # zero stationary: bank-init matmul writes 0 everywhere and sets
            # every has_written bit, making all later flags=0 matmuls pure
            # accumulates regardless of scheduler order
            zstat = consts.tile([128, 128], BF16, tag="zstat")
            nc.vector.memset(zstat[:], 0.0)
            # ss stationary: sstat[:, u, r] = 1 iff r == u (tile parity u)
            sstat = consts.tile([128, 2, 2], BF16, tag="sstat")
            nc.vector.memset(sstat[:], 0.0)
            for u in range(2):
                nc.vector.memset(sstat[:, u, u : u + 1], 1.0)
            # sel stationary: rcsel[32g+k, u, r] = rc_k iff r == u
            rcsel = consts.tile([128, 2, 2], BF16, tag="rcsel")
            nc.vector.memset(rcsel[:], 0.0)

            # center copies + csq on GPSIMD so the DVE queue never blocks on
            # the ct DMA and rc is ready early (first ACT op -> one table set)
            ctb = consts.tile([128, 2, NCLS], BF16, tag="ctb")
            nc.vector.tensor_copy(ctb[:], ct_in[:])
            csq = consts.tile([128, 2, NCLS], BF16, tag="csq")
            nc.vector.tensor_mul(out=csq[:], in0=ctb[:], in1=ctb[:])

            # ---- PE warm-up (HAM unthrottle): independent of any DMA ----
            warm = dotsp.tile([128, GRP], F32, tag="dots")
            for _ in range(8):
                nc.tensor.matmul(
                    warm[0:NCLS, :], wmov[:, 0:NCLS], wmov[:], start=True, stop=True
                )
            # center norms: rc[k] = 1/||C_k||
            ssc = accps.tile([NCLS, 1], F32, tag="ssc")
            nc.tensor.matmul(ssc[:], csq[:, 0, :], ones_b[:], start=True, stop=False)
            nc.tensor.matmul(ssc[:], csq[:, 1, :], ones_b[:], start=False, stop=True)
            for _ in range(4):
                nc.tensor.matmul(
                    warm[0:NCLS, :], wmov[:, 0:NCLS], wmov[:], start=True, stop=True
                )
            rc = consts.tile([NCLS, 1], F32, tag="rc")
            nc.scalar.activation(out=rc[:], in_=ssc[:], func=AF.Abs_reciprocal_sqrt)
            for g in range(4):
                for u in range(2):
                    nc.vector.tensor_copy(
                        rcsel[32 * g : 32 * g + NCLS, u, u : u + 1], rc[:]
                    )

            # ---- accumulators: bank 0 <- tiles 0,1; bank 1 <- tiles 2,3 ----
            ss_b = [
                accps.tile([128, GRP], F32, tag=f"ss{b}", name=f"ss{b}")
                for b in range(2)
            ]
            sel_b = [
                accps.tile([128, GRP], F32, tag=f"sel{b}", name=f"sel{b}")
                for b in range(2)
            ]
            for b in range(2):
                nc.tensor.matmul(
                    ss_b[b][:], zstat[:], wmov[:], start=True, stop=False
                )
                nc.tensor.matmul(
                    sel_b[b][:], zstat[:], wmov[:], start=True, stop=False
                )

            pending_sel = []

            def emit_sel(t, ps4):
                b, u = t >> 1, t & 1
                for g in range(4):
                    nc.tensor.matmul(
                        sel_b[b][32 * g : 32 * g + 2, :],
                        rcsel[32 * g : 32 * g + NCLS, u, :],
                        ps4[32 * g : 32 * g + NCLS, :],
                        start=False,
                        stop=(u == 1 and g == 3),
                        tile_position=(32 * g, 32 * g),
                    )

            def emit_finish(b):
                rsq = accp.tile([128, GRP], F32, tag=f"rsq{b}", name=f"rsq{b}")
                nc.scalar.activation(
                    out=rsq[:], in_=ss_b[b][:], func=AF.Abs_reciprocal_sqrt
                )
                acc = accp.tile([128, GRP], F32, tag=f"acc{b}", name=f"acc{b}")
                nc.vector.tensor_mul(out=acc[:], in0=rsq[:], in1=sel_b[b][:])
                partial = accp.tile(
                    [128, 1], F32, tag=f"partial{b}", name=f"partial{b}"
                )
                nc.vector.tensor_reduce(
                    out=partial[:],
                    in_=acc[:],
                    axis=mybir.AxisListType.X,
                    op=mybir.AluOpType.add,
                )
                nc.sync.dma_start(out=out_d[b], in_=partial[:])

            # ---- main loop ----
            for t in range(N_TILES):
                b, u = t >> 1, t & 1
                xt = xts[t]
                xsqt = xsqp.tile([128, 2, TILE_F], BF16, tag="xsqt")
                nc.scalar.activation(
                    out=xsqt[:, 0, :], in_=xt[:, 0, :], func=AF.Square
                )
                nc.vector.tensor_mul(
                    out=xsqt[:, 1, :], in0=xt[:, 1, :], in1=xt[:, 1, :]
                )

                dots4 = dotsp.tile([128, GRP], F32, tag="dots")
                nc.tensor.matmul(
                    dots4[:], zstat[:], wmov[:], start=True, stop=False
                )
                for g in range(4):
                    gsl = slice(g * GRP, (g + 1) * GRP)
                    for a in range(2):
                        nc.tensor.matmul(
                            dots4[32 * g : 32 * g + NCLS, :],
                            ctb[:, a, :],
                            xt[:, a, gsl],
                            start=False,
                            stop=(g == 3 and a == 1),
                            tile_position=(0, 32 * g),
                        )
                ps4 = small.tile([128, GRP], BF16, tag="ps4")
                nc.vector.tensor_mul(out=ps4[:], in0=oh4[:, t, :], in1=dots4[:])
                pending_sel.append((t, ps4))

                for g in range(4):
                    gsl = slice(g * GRP, (g + 1) * GRP)
                    for a in range(2):
                        nc.tensor.matmul(
                            ss_b[b][32 * g : 32 * g + 2, :],
                            sstat[:, u, :],
                            xsqt[:, a, gsl],
                            start=False,
                            stop=(u == 1 and g == 3 and a == 1),
                            tile_position=(0, 32 * g),
                        )

                # sel for the previous tile (lag 1 so PE never waits on DVE)
                if len(pending_sel) > 1:
                    emit_sel(*pending_sel.pop(0))
                if t == 2:
                    emit_finish(0)  # bank A finish overlaps tiles 2-3
            emit_sel(*pending_sel.pop(0))
            emit_finish(1)

    nc.compile()
    return nc


def shard_inputs(x, centers, labels):
    """Full inputs -> list of 8 per-core input maps."""
    import ml_dtypes

    fp8_np = mybir.dt.np(FP8)
    x = np.asarray(x, dtype=np.float32)
    centers = np.asarray(centers, dtype=np.float32)
    labels = np.asarray(labels)

    xr = x.reshape(N_IMG, C, 2, PIX_PER_CORE)
    labr = labels.reshape(N_IMG, 2, PIX_PER_CORE)
    ctr = np.ascontiguousarray(
        centers.T.astype(ml_dtypes.bfloat16).reshape(2, 128, NCLS).transpose(1, 0, 2)
    )
    kvals = np.arange(NCLS, dtype=np.int64)

    in_maps = []
    for core in range(N_CORES):
        n, j = core // 2, core % 2
        xc = xr[n, :, j, :].astype(ml_dtypes.bfloat16).reshape(
            2, 128, N_TILES, TILE_F
        )
        xs = np.ascontiguousarray(xc.transpose(2, 1, 0, 3))
        # onehot4[32g+k, t, p] = (label[t*2048 + g*512 + p] == k)
        lab4 = labr[n, j].reshape(N_TILES, 4, GRP)
        oh4 = np.zeros((128, N_TILES, GRP), dtype=fp8_np)
        for g in range(4):
            oh4[32 * g : 32 * g + NCLS] = (
                lab4[:, g, None, :] == kvals[None, :, None]
            ).astype(fp8_np).transpose(1, 0, 2)
        in_maps.append(
            {"x": xs, "onehot4": np.ascontiguousarray(oh4), "centersT": ctr}
        )
    return in_maps


_NC_CACHE = {}


def _ensure_ntff_hook():
    """Register the axon NTFF profile hook if the optional antenv.axon_hooks
    module is absent from this image (bass_utils hard-imports it when
    trace=True)."""
    try:
        from antenv.axon_hooks import get_axon_ntff_profile_hook  # noqa: F401

        return
    except ImportError:
        pass
    import types

    import antenv

    mod = types.ModuleType("antenv.axon_hooks")
    state = {"hook": None}
    mod.set_axon_ntff_profile_hook = lambda h: state.__setitem__("hook", h)
    mod.get_axon_ntff_profile_hook = lambda: state["hook"]
    sys.modules["antenv.axon_hooks"] = mod
    antenv.axon_hooks = mod
    try:
        from trn_agent_boot.trn_boot import _ntff_profile_via_ctypes

        mod.set_axon_ntff_profile_hook(
            _ntff_profile_via_ctypes("/opt/axon/libaxon_pjrt.so")
        )
    except Exception:
        pass


# live output partitions: ss/sel rows sit at partition 32g+u
_LIVE = [32 * g + u for g in range(4) for u in range(2)]


def kernel(x, centers, labels, _profile=False):
    in_maps = shard_inputs(x, centers, labels)
    if _profile:
        _ensure_ntff_hook()
    if "nc" not in _NC_CACHE:
        _NC_CACHE["nc"] = build_nc()
    nc = _NC_CACHE["nc"]
    res = run_bass_kernel_spmd(
        nc, in_maps, list(range(N_CORES)), trace=bool(_profile)
    )
    s = 0.0
    for r in res.results:
        o = np.asarray(r["out"], dtype=np.float64)  # [2, 128, 1]
        s += o[:, _LIVE, 0].sum()
    val = np.array(np.float32(2.0 - 2.0 * s / PIX_TOTAL))
    if _profile:
        return val, res
    return val


# revision 29
# speedup vs baseline: 1.1222x; 1.1222x over previous
"""CenterLoss kernel for Trainium2 (8 NeuronCores, SPMD data-parallel).

Math: for pixel p with feature x_p (256-ch), label l_p, centers C[19,256]:
    dist_p = ||xn_p||^2 + ||cn_{l_p}||^2 - 2 * xn_p . cn_{l_p}
with xn = x/||x||, cn = C/||C|| (row-wise).  ||xn||^2 == ||cn||^2 == 1 up to
f32 rounding, so   mean(dist) = 2 - (2/B) * S,  S = sum_p (x_p.cn_{l_p})/||x_p||.

Device pipeline (per 2048-px tile, 4 col-tiled 512-px groups g in parallel
32-column strips of the PE array -- every output here is <=32 partitions
wide, so 4 matmuls with different moving operands run concurrently):
  - dots4[32g+k, p] = ct.T @ x_g            (PE, strip g; chunk0 fp8 +
                     chunk1 bf16 accumulated in PSUM f32)
  - prodsel4       = onehot4 * dots4        (DVE, one op per tile; onehot4
                     is staged host-side in the same 32g+k partition layout)
  - sel[32g+u, p] += rc-col.T @ prodsel4_g  (PE, strip (g,g); u = tile parity;
                     stationary [19,2] col u = rc = 1/||C_k||)
  - ss[32g+u, p]  += ones-col.T @ xsq_g     (PE, strip g)
  - xsq: three engines in parallel per tile: ACT Square on fp8 chunk0 halves
    0:1024, GPSIMD mul on chunk0 1024:2048, DVE mul on bf16 chunk1 (2x mode)
  - finish per bank (tiles 01 -> bank A, 23 -> B, A overlapped with compute):
    out = sum_p sel * rsqrt(ss)   (ACT Abs_reciprocal_sqrt; same act table
    set as Square so only one ACT_TABLE_LOAD)

PSUM has_written discipline: start=True clears the WHOLE bank's bits, so
each bank gets exactly one start=True on its first matmul; all later
matmuls use flags=0 (overwrite-if-clear / accumulate-if-set per element).

DMA: x chunk0 (fp8, 1MiB) tiles on the sync HWDGE ring; onehot4 + x chunk1
(bf16, 2MiB) tiles on the scalar HWDGE ring (parallel issue).  Everything
is staged partition-major contiguous (>=2KB per-partition runs).

Sharding: 65536 pixels -> 8 cores x 8192 (core c: image c//2, half c%2).
Host sums the live partitions (32g+u) of the [2,128,1] output.
"""

import sys

import numpy as np

if "/opt/trn_rl_repo" not in sys.path:
    sys.path.insert(0, "/opt/trn_rl_repo")

import concourse.bacc as bacc
import concourse.tile as tile
from concourse import mybir
from concourse.bass_utils import run_bass_kernel_spmd

N_CORES = 8
C = 256
NCLS = 19
N_IMG, H, W = 4, 128, 128
PIX_TOTAL = N_IMG * H * W          # 65536
PIX_PER_CORE = PIX_TOTAL // N_CORES  # 8192
TILE_F = 2048                      # pixels per DMA tile
N_TILES = PIX_PER_CORE // TILE_F   # 4
GRP = 512                          # pixels per col-strip group
F32 = mybir.dt.float32
BF16 = mybir.dt.bfloat16
FP8 = mybir.dt.float8e4


def build_nc():
    """Build the per-core Bass program (same program on all 8 cores)."""
    AF = mybir.ActivationFunctionType

    import ml_dtypes

    nc = bacc.Bacc(None, target_bir_lowering=False, debug=False)
    x_d = nc.dram_tensor("x", [N_TILES, 128, 2, TILE_F], BF16, kind="ExternalInput")
    oh_d = nc.dram_tensor("onehot4", [128, N_TILES, GRP], FP8, kind="ExternalInput")
    ct_d = nc.dram_tensor("centersT", [128, 2, NCLS], BF16, kind="ExternalInput")
    out_d = nc.dram_tensor("out", [2, 128, 1], F32, kind="ExternalOutput")

    with tile.TileContext(nc) as tc:
        with (
            tc.tile_pool(name="consts", bufs=1) as consts,
            tc.tile_pool(name="x0in", bufs=4) as x0in,
            tc.tile_pool(name="x1in", bufs=4) as x1in,
            tc.tile_pool(name="xsq", bufs=2) as xsqp,
            tc.tile_pool(name="small", bufs=2) as small,
            tc.tile_pool(name="accum", bufs=1) as accp,
            tc.tile_pool(name="dots", bufs=3, space="PSUM") as dotsp,
            tc.tile_pool(name="acc_ps", bufs=1, space="PSUM") as accps,
        ):
            # ---- DMAs: byte-balanced across the two HWDGE rings in tile
            # consumption order (each ring is one FIFO queue; the SDMA
            # engines round-robin between the two queues) ----
            ct_in = consts.tile([128, 2, NCLS], BF16, tag="ct_in")
            nc.sync.dma_start(out=ct_in[:], in_=ct_d[:])
            oh4 = consts.tile([128, N_TILES, GRP], FP8, tag="oh4")
            nc.gpsimd.dma_start(out=oh4[:], in_=oh_d[:])
            xts = []
            for t in range(N_TILES):
                xt = x0in.tile([128, 2, TILE_F], BF16, tag="xt", name=f"xt{t}")
                nc.sync.dma_start(out=xt[:], in_=x_d[t])
                xts.append(xt)

            # ---- constants ----
            # wmov first: the PE warmup matmuls depend only on it.
            wmov = consts.tile([128, GRP], BF16, tag="wmov")
            nc.vector.memset(wmov[:], 0.5)
            ones_b = consts.tile([128, 1], BF16, tag="ones_b")
            nc.vector.memset(ones_b[:], 1.0)
            # zero stationary: bank-init matmul writes 0 everywhere and sets
            # every has_written bit, making all later flags=0 matmuls pure
            # accumulates regardless of scheduler order
            zstat = consts.tile([128, 128], BF16, tag="zstat")
            nc.vector.memset(zstat[:], 0.0)
            # ss stationary: sstat[:, u, r] = 1 iff r == u (tile parity u)
            sstat = consts.tile([128, 2, 2], BF16, tag="sstat")
            nc.vector.memset(sstat[:], 0.0)
            for u in range(2):
                nc.vector.memset(sstat[:, u, u : u + 1], 1.0)
            # sel stationary: rcsel[32g+k, u, r] = rc_k iff r == u
            rcsel = consts.tile([128, 2, 2], BF16, tag="rcsel")
            nc.vector.memset(rcsel[:], 0.0)

            # center copies + csq on GPSIMD so the DVE queue never blocks on
            # the ct DMA and rc is ready early (first ACT op -> one table set)
            ctb = consts.tile([128, 2, NCLS], BF16, tag="ctb")
            nc.vector.tensor_copy(ctb[:], ct_in[:])
            csq = consts.tile([128, 2, NCLS], BF16, tag="csq")
            nc.vector.tensor_mul(out=csq[:], in0=ctb[:], in1=ctb[:])

            # ---- PE warm-up (HAM unthrottle): independent of any DMA ----
            warm = dotsp.tile([128, GRP], F32, tag="dots")
            for _ in range(8):
                nc.tensor.matmul(
                    warm[0:NCLS, :], wmov[:, 0:NCLS], wmov[:], start=True, stop=True
                )
            # center norms: rc[k] = 1/||C_k||
            ssc = accps.tile([NCLS, 1], F32, tag="ssc")
            nc.tensor.matmul(ssc[:], csq[:, 0, :], ones_b[:], start=True, stop=False)
            nc.tensor.matmul(ssc[:], csq[:, 1, :], ones_b[:], start=False, stop=True)
            for _ in range(4):
                nc.tensor.matmul(
                    warm[0:NCLS, :], wmov[:, 0:NCLS], wmov[:], start=True, stop=True
                )
            rc = consts.tile([NCLS, 1], F32, tag="rc")
            nc.scalar.activation(out=rc[:], in_=ssc[:], func=AF.Abs_reciprocal_sqrt)
            for g in range(4):
                for u in range(2):
                    nc.vector.tensor_copy(
                        rcsel[32 * g : 32 * g + NCLS, u, u : u + 1], rc[:]
                    )

            # ---- accumulators: bank 0 <- tiles 0,1; bank 1 <- tiles 2,3 ----
            ss_b = [
                accps.tile([128, GRP], F32, tag=f"ss{b}", name=f"ss{b}")
                for b in range(2)
            ]
            sel_b = [
                accps.tile([128, GRP], F32, tag=f"sel{b}", name=f"sel{b}")
                for b in range(2)
            ]
            for b in range(2):
                nc.tensor.matmul(
                    ss_b[b][:], zstat[:], wmov[:], start=True, stop=False
                )
                nc.tensor.matmul(
                    sel_b[b][:], zstat[:], wmov[:], start=True, stop=False
                )

            pending_sel = []

            def emit_sel(t, ps4):
                b, u = t >> 1, t & 1
                for g in range(4):
                    nc.tensor.matmul(
                        sel_b[b][32 * g : 32 * g + 2, :],
                        rcsel[32 * g : 32 * g + NCLS, u, :],
                        ps4[32 * g : 32 * g + NCLS, :],
                        start=False,
                        stop=(u == 1 and g == 3),
                        tile_position=(32 * g, 32 * g),
                    )

            def emit_finish(b):
                rsq = accp.tile([128, GRP], F32, tag=f"rsq{b}", name=f"rsq{b}")
                nc.scalar.activation(
                    out=rsq[:], in_=ss_b[b][:], func=AF.Abs_reciprocal_sqrt
                )
                acc = accp.tile([128, GRP], F32, tag=f"acc{b}", name=f"acc{b}")
                nc.vector.tensor_mul(out=acc[:], in0=rsq[:], in1=sel_b[b][:])
                partial = accp.tile(
                    [128, 1], F32, tag=f"partial{b}", name=f"partial{b}"
                )
                nc.vector.tensor_reduce(
                    out=partial[:],
                    in_=acc[:],
                    axis=mybir.AxisListType.X,
                    op=mybir.AluOpType.add,
                )
                nc.sync.dma_start(out=out_d[b], in_=partial[:])

            # ---- main loop ----
            for t in range(N_TILES):
                b, u = t >> 1, t & 1
                xt = xts[t]
                xsqt = xsqp.tile([128, 2, TILE_F], BF16, tag="xsqt")
                nc.scalar.activation(
                    out=xsqt[:, 0, :], in_=xt[:, 0, :], func=AF.Square
                )
                nc.vector.tensor_mul(
                    out=xsqt[:, 1, :], in0=xt[:, 1, :], in1=xt[:, 1, :]
                )

                dots4 = dotsp.tile([128, GRP], F32, tag="dots")
                nc.tensor.matmul(
                    dots4[:], zstat[:], wmov[:], start=True, stop=False
                )
                for g in range(4):
                    gsl = slice(g * GRP, (g + 1) * GRP)
                    for a in range(2):
                        nc.tensor.matmul(
                            dots4[32 * g : 32 * g + NCLS, :],
                            ctb[:, a, :],
                            xt[:, a, gsl],
                            start=False,
                            stop=(g == 3 and a == 1),
                            tile_position=(0, 32 * g),
                        )
                ps4 = small.tile([128, GRP], BF16, tag="ps4")
                nc.vector.tensor_mul(out=ps4[:], in0=oh4[:, t, :], in1=dots4[:])
                pending_sel.append((t, ps4))

                for g in range(4):
                    gsl = slice(g * GRP, (g + 1) * GRP)
                    for a in range(2):
                        nc.tensor.matmul(
                            ss_b[b][32 * g : 32 * g + 2, :],
                            sstat[:, u, :],
                            xsqt[:, a, gsl],
                            start=False,
                            stop=(u == 1 and g == 3 and a == 1),
                            tile_position=(0, 32 * g),
                        )

                # sel for the previous tile (lag 1 so PE never waits on DVE)
                if len(pending_sel) > 1:
                    emit_sel(*pending_sel.pop(0))
                if t == 2:
                    emit_finish(0)  # bank A finish overlaps tiles 2-3
            emit_sel(*pending_sel.pop(0))
            emit_finish(1)

    nc.compile()
    return nc


def shard_inputs(x, centers, labels):
    """Full inputs -> list of 8 per-core input maps."""
    import ml_dtypes

    fp8_np = mybir.dt.np(FP8)
    x = np.asarray(x, dtype=np.float32)
    centers = np.asarray(centers, dtype=np.float32)
    labels = np.asarray(labels)

    xr = x.reshape(N_IMG, C, 2, PIX_PER_CORE)
    labr = labels.reshape(N_IMG, 2, PIX_PER_CORE)
    ctr = np.ascontiguousarray(
        centers.T.astype(ml_dtypes.bfloat16).reshape(2, 128, NCLS).transpose(1, 0, 2)
    )
    kvals = np.arange(NCLS, dtype=np.int64)

    in_maps = []
    for core in range(N_CORES):
        n, j = core // 2, core % 2
        xc = xr[n, :, j, :].astype(ml_dtypes.bfloat16).reshape(
            2, 128, N_TILES, TILE_F
        )
        xs = np.ascontiguousarray(xc.transpose(2, 1, 0, 3))
        # onehot4[32g+k, t, p] = (label[t*2048 + g*512 + p] == k)
        lab4 = labr[n, j].reshape(N_TILES, 4, GRP)
        oh4 = np.zeros((128, N_TILES, GRP), dtype=fp8_np)
        for g in range(4):
            oh4[32 * g : 32 * g + NCLS] = (
                lab4[:, g, None, :] == kvals[None, :, None]
            ).astype(fp8_np).transpose(1, 0, 2)
        in_maps.append(
            {"x": xs, "onehot4": np.ascontiguousarray(oh4), "centersT": ctr}
        )
    return in_maps


_NC_CACHE = {}


def _ensure_ntff_hook():
    """Register the axon NTFF profile hook if the optional antenv.axon_hooks
    module is absent from this image (bass_utils hard-imports it when
    trace=True)."""
    try:
        from antenv.axon_hooks import get_axon_ntff_profile_hook  # noqa: F401

        return
    except ImportError:
        pass
    import types

    import antenv

    mod = types.ModuleType("antenv.axon_hooks")
    state = {"hook": None}
    mod.set_axon_ntff_profile_hook = lambda h: state.__setitem__("hook", h)
    mod.get_axon_ntff_profile_hook = lambda: state["hook"]
    sys.modules["antenv.axon_hooks"] = mod
    antenv.axon_hooks = mod
    try:
        from trn_agent_boot.trn_boot import _ntff_profile_via_ctypes

        mod.set_axon_ntff_profile_hook(
            _ntff_profile_via_ctypes("/opt/axon/libaxon_pjrt.so")
        )
    except Exception:
        pass


# live output partitions: ss/sel rows sit at partition 32g+u
_LIVE = [32 * g + u for g in range(4) for u in range(2)]


def kernel(x, centers, labels, _profile=False):
    in_maps = shard_inputs(x, centers, labels)
    if _profile:
        _ensure_ntff_hook()
    if "nc" not in _NC_CACHE:
        _NC_CACHE["nc"] = build_nc()
    nc = _NC_CACHE["nc"]
    res = run_bass_kernel_spmd(
        nc, in_maps, list(range(N_CORES)), trace=bool(_profile)
    )
    s = 0.0
    for r in res.results:
        o = np.asarray(r["out"], dtype=np.float64)  # [2, 128, 1]
        s += o[:, _LIVE, 0].sum()
    val = np.array(np.float32(2.0 - 2.0 * s / PIX_TOTAL))
    if _profile:
        return val, res
    return val
